# revision 52
# baseline (speedup 1.0000x reference)
"""3-layer GAT on 8 TRN2 NeuronCores (v3).

Node-sharded 8 ways (2500/core padded to 2560). Per layer, a global node
table holds [h | al_s | al_d] rows; edges (sorted by dst block, padded to
Q*128 per block) gather src rows and aggregate via one-hot matmuls.

- Layer-1 table: replicated compute (no collective) — every core runs the
  feature matmul for all 160 global chunks from the replicated fp8 x input.
- Tables are fp8 rows (h fp8, al bf16 packed in trailing bytes): 512B rows
  for layers 1/2 (256 fp8 h + 16B al), 256B rows for layer 3 (64 bf16 h +
  4B al). Gathers fetch one row per edge.
- al_d per edge: Q mini-matmuls per block against host-precomputed
  transposed one-hot chunks (fp8, SBUF-cached; stationary loads are free).
- Aggregation: per 128-edge chunk, matmul(one-hot fp8, [w*h | w] bf16) with
  PSUM accumulation; segment softmax denominator rides in the last cols.
- Layers 2/3 tables: next-layer feature matmul interleaved per-block into
  the edge phase; rows AllGathered compactly in 3 regions (region-major
  global layout keeps outputs contiguous), then re-strided locally to
  gather-legal 256B-multiple row pitch. Collectives overlap edge compute.
"""
import os
import sys
sys.path.insert(0, "/opt/trn_rl_repo")
import numpy as np
import ml_dtypes

import concourse.bass as bass
import concourse.tile as tile
from concourse import bacc, mybir
from concourse.bass_utils import run_bass_kernel_spmd
from concourse.masks import make_identity

BF16 = ml_dtypes.bfloat16
FP8 = ml_dtypes.float8_e4m3fn
N = 20000
E = 320000
FIN = 1024
H = 4
C = 64
NEG = 0.2
NCORE = 8
NLOC = 2500
NPAD = 2560           # per-core node rows, padded to x128
NBLK = NPAD // 128    # dst blocks per core
NG = NCORE * NPAD     # global table rows
GBLK = NG // 128      # global 128-chunks
RB12 = 512            # L1/L2 row bytes: h fp8 0:256, al bf16 256:272
CB12 = 272            # compact row bytes (AllGather payload)
RB3 = 256             # L3 row bytes: h bf16 0:128, al bf16 128:132
CB3 = 132
DVE_HEADS = 3         # heads 0:k multiplied on DVE, rest on GPSIMD
AG_SPLITS = [640, 1280, 1920]  # chunked-AllGather region bounds (local rows)

_cache = {}


def _regions():
    bounds = [0] + AG_SPLITS + [NPAD]
    regs = []
    base = 0
    for lo, hi in zip(bounds[:-1], bounds[1:]):
        regs.append((lo, hi, base))
        base += NCORE * (hi - lo)
    return regs


def _rowmap(core, loc):
    """Global table row id for node (core, loc) under the region layout
    [region, core, rows] — keeps chunked AllGather outputs contiguous."""
    core, loc = np.broadcast_arrays(np.asarray(core, np.int64),
                                    np.asarray(loc, np.int64))
    out = np.empty_like(loc)
    for lo, hi, base in _regions():
        m = (loc >= lo) & (loc < hi)
        out[m] = base + core[m] * (hi - lo) + (loc[m] - lo)
    return out


def _prep_edges(src, dst):
    """Partition edges by dst core, sort by (dst block, src), pad per block.

    Returns (Q, per_core) with per_core[c] = (idx_s [NBLK, Q*128] global row
    ids, dst_rel [NBLK, Q*128] int32, -1 for pad)."""
    core = dst // NLOC
    dloc = dst - core * NLOC
    blk = dloc // 128
    per_core_lists = []
    maxq = 0
    for c in range(NCORE):
        m = core == c
        s_c, dl_c, b_c = src[m], dloc[m], blk[m]
        order = np.lexsort((s_c, b_c))
        s_c, dl_c, b_c = s_c[order], dl_c[order], b_c[order]
        counts = np.bincount(b_c, minlength=NBLK)
        maxq = max(maxq, int(np.ceil(counts.max() / 128)))
        per_core_lists.append((s_c, dl_c, counts))
    Q = maxq
    EPB = Q * 128
    out = []
    for c in range(NCORE):
        s_c, dl_c, counts = per_core_lists[c]
        idx_s = np.zeros((NBLK, EPB), np.int64)
        dst_rel = np.full((NBLK, EPB), -1, np.int32)
        pos = 0
        for b in range(NBLK):
            n = counts[b]
            sb = s_c[pos:pos + n]
            db = dl_c[pos:pos + n]
            pos += n
            idx_s[b, :n] = _rowmap(sb // NLOC, sb % NLOC)
            dst_rel[b, :n] = db % 128
        out.append((idx_s, dst_rel))
    return Q, out


def _wrap_idx(idx_flat):
    """flat [n] -> dma_gather idx layout [128, n//16] int16."""
    n = idx_flat.shape[0]
    w = idx_flat.reshape(n // 16, 16).T.astype(np.int16)
    return np.tile(w, (8, 1))


def _onehots(dst_rel, Q):
    """dst_rel [NBLK, Q*128] -> (sblk, sblkT) [128, NBLK*Q*128] fp8."""
    r = dst_rel.reshape(NBLK, Q, 128)              # [b, s, p]
    eye = np.eye(128, dtype=FP8)
    lut = np.concatenate([eye, np.zeros((1, 128), FP8)], axis=0)
    oh = lut[np.where(r < 0, 128, r)]              # [b, s, p, d]
    sblk = np.ascontiguousarray(oh.transpose(2, 0, 1, 3)).reshape(128, -1)
    sblkT = np.ascontiguousarray(oh.transpose(3, 0, 1, 2)).reshape(128, -1)
    return sblk, sblkT


def _fold_w(W, a_s, a_d):
    """[F, H*C] weights + per-head a vectors -> [F, H*C + 2H] f32."""
    F = W.shape[0]
    Hh, Cc = a_s.shape
    As = np.zeros((Hh * Cc, Hh), np.float64)
    Ad = np.zeros((Hh * Cc, Hh), np.float64)
    for h in range(Hh):
        As[h * Cc:(h + 1) * Cc, h] = a_s[h]
        Ad[h * Cc:(h + 1) * Cc, h] = a_d[h]
    W64 = W.astype(np.float64)
    return np.concatenate([W64, W64 @ As, W64 @ Ad], axis=1).astype(np.float32)


def _build(Q):
    dt = mybir.dt
    nc = bacc.Bacc("TRN2", num_devices=NCORE, debug=False, num_swdge_queues=4)
    NBQ = NBLK * Q

    # x^T replicated, fp8, pre-chunked: [p, (g k n)] = xT[k*128+p, col(g*128+n)]
    xtg_in = nc.dram_tensor("xtg", [128, GBLK * 8 * 128], dt.bfloat16,
                            kind="ExternalInput")
    w1e_in = nc.dram_tensor("w1e", [FIN, 264], dt.bfloat16, kind="ExternalInput")
    w2e_in = nc.dram_tensor("w2e", [256, 264], dt.bfloat16, kind="ExternalInput")
    w3e_in = nc.dram_tensor("w3e", [256, 6], dt.bfloat16, kind="ExternalInput")

    b1_in = nc.dram_tensor("b1r", [128, 256], dt.bfloat16, kind="ExternalInput")
    b2_in = nc.dram_tensor("b2r", [128, 256], dt.bfloat16, kind="ExternalInput")

    bc_in = nc.dram_tensor("bcr", [128, 4], dt.float32, kind="ExternalInput")
    ixs_in = nc.dram_tensor("ixs", [128, NBQ * 8], dt.int16, kind="ExternalInput")
    ixd_in = nc.dram_tensor("ixd", [128, NPAD // 16], dt.int16, kind="ExternalInput")
    sblk_in = nc.dram_tensor("sblk", [128, NBQ * 128], dt.float8e4,
                             kind="ExternalInput")
    sblkT_in = nc.dram_tensor("sblkT", [128, NBQ * 128], dt.float8e4,
                              kind="ExternalInput")
    out_d = nc.dram_tensor("out", [NPAD, 4], dt.float32, kind="ExternalOutput")

    tab1 = nc.dram_tensor("tab1", [NG, 384], dt.bfloat16, kind="Internal")
    tin2 = nc.dram_tensor("tin2", [NPAD, CB12], dt.float8e4, kind="Internal")
    tabc2 = nc.dram_tensor("tabc2", [NG, CB12], dt.float8e4, kind="Internal",
                           addr_space="Shared")
    tabg2 = nc.dram_tensor("tabg2", [NG, RB12], dt.float8e4, kind="Internal")
    tin3 = nc.dram_tensor("tin3", [NPAD, 6], dt.bfloat16, kind="Internal")
    tabc3 = nc.dram_tensor("tabc3", [NG, 6], dt.bfloat16, kind="Internal",
                           addr_space="Shared")
    tabg3 = nc.dram_tensor("tabg3", [NG, 128], dt.bfloat16, kind="Internal")

    with tile.TileContext(nc) as tc:
        with (
            tc.tile_pool(name="const", bufs=1) as cpool,
            tc.tile_pool(name="work", bufs=2) as wpool,
        ):
            # ---- constants to SBUF
            ident = cpool.tile([128, 128], dt.bfloat16)
            make_identity(nc, ident[:])
            ixs = cpool.tile([128, NBQ * 8], dt.int16)
            ixd = cpool.tile([128, NPAD // 16], dt.int16)
            nc.sync.dma_start(out=ixd[:], in_=ixd_in[:])
            sblk = cpool.tile([128, NBQ * 128], dt.float8e4)
            sblkT8 = cpool.tile([128, NBQ * 128], dt.float8e4)
            w1e = cpool.tile([128, 8, 264], dt.bfloat16)
            nc.sync.dma_start(out=w1e[:], in_=w1e_in[:].rearrange("(k p) c -> p k c", p=128))
            w2e = cpool.tile([128, 2, 264], dt.bfloat16)
            nc.sync.dma_start(out=w2e[:], in_=w2e_in[:].rearrange("(k p) c -> p k c", p=128))
            w3e = cpool.tile([128, 2, 6], dt.bfloat16)
            nc.sync.dma_start(out=w3e[:], in_=w3e_in[:].rearrange("(k p) c -> p k c", p=128))

            b1r = cpool.tile([128, 256], dt.bfloat16)
            nc.sync.dma_start(out=b1r[:], in_=b1_in[:])
            b2r = cpool.tile([128, 256], dt.bfloat16)
            nc.sync.dma_start(out=b2r[:], in_=b2_in[:])

            bcr = cpool.tile([128, 4], dt.float32)
            nc.sync.dma_start(out=bcr[:], in_=bc_in[:])

            alD1 = cpool.tile([128, NBLK, 128], dt.bfloat16)
            alD2 = cpool.tile([128, NBLK, H], dt.bfloat16)
            alD3 = cpool.tile([128, NBLK, 1], dt.bfloat16)
            xt2 = cpool.tile([128, 2, NPAD], dt.bfloat16)
            xt2b = xt2  # lifetimes don't overlap; tile deps order the reuse


            # ===== layer 1: replicated phase_a, batched x4
            GB = 4
            with tc.tile_pool(name="psA", bufs=2, space="PSUM") as ppA:
                for gg in range(0, GBLK, GB):
                    xc = wpool.tile([128, GB, 8, 128], dt.bfloat16, tag="xc")
                    nc.sync.dma_start(
                        out=xc[:],
                        in_=xtg_in[:, gg * 1024:(gg + GB) * 1024]
                        .rearrange("p (j k n) -> p j k n", k=8, n=128))
                    hrow = wpool.tile([128, GB, 264], dt.bfloat16, tag="hrow")
                    for j in range(GB):
                        ps = ppA.tile([128, 264], dt.float32, tag=f"psA{j % 2}")
                        for k in range(8):
                            nc.tensor.matmul(ps[:], xc[:, j, k, :], w1e[:, k, :],
                                             start=(k == 0), stop=(k == 7))
                        if j % 2 == 0:
                            nc.scalar.activation(hrow[:, j, :], ps[:],
                                                 mybir.ActivationFunctionType.Copy)
                        else:
                            nc.vector.tensor_copy(hrow[:, j, :], ps[:])
                    nc.sync.dma_start(
                        out=tab1[gg * 128:(gg + GB) * 128, 0:264]
                        .rearrange("(j p) c -> p j c", p=128),
                        in_=hrow[:])
            # one-hot caches + edge indices load under the L1 compute
            nc.sync.dma_start(out=ixs[:], in_=ixs_in[:])
            nc.sync.dma_start(out=sblk[:], in_=sblk_in[:])
            nc.sync.dma_start(out=sblkT8[:], in_=sblkT_in[:])
            # own nodes' rows for al_d: one gather
            nc.gpsimd.dma_gather(
                out_ap=alD1[:], in_ap=tab1[:, 256:384], idxs_ap=ixd[:],
                num_idxs=NPAD, num_idxs_reg=NPAD, elem_size=128,
                elem_step=384, single_packet=False, queue_num=0)

            def edge_phase(tab, rb, tdt, hcols, hh, h_view, als_view, ald_ap,
                           brep, do_relu, xt_out, post_block):
                """Per-dst-block edge processing. h_view/als_view map the raw
                gathered fp8-byte tile to h and al_s APs; ald_ap(b) gives the
                block's own al_d [128, hh] bf16."""
                mcols = hcols + hh
                with tc.tile_pool(name="psE", bufs=2, space="PSUM") as pp:
                    for b in range(NBLK):
                        g = wpool.tile([128, Q, rb], tdt, tag="g", bufs=3)
                        nc.gpsimd.dma_gather(
                            out_ap=g[:], in_ap=tab[:],
                            idxs_ap=ixs[:, b * Q * 8:(b + 1) * Q * 8],
                            num_idxs=Q * 128, num_idxs_reg=Q * 128,
                            elem_size=rb, single_packet=False,
                            queue_num=b % 4)
                        gw = wpool.tile([128, Q, mcols], dt.bfloat16, tag="gw")
                        # al_d per edge via mini-matmuls vs transposed one-hots
                        ps_e = pp.tile([128, Q, hh], dt.float32, tag="ps_e")
                        for s in range(Q):
                            nc.tensor.matmul(
                                ps_e[:, s, :],
                                sblkT8[:, (b * Q + s) * 128:(b * Q + s + 1) * 128],
                                ald_ap(b), start=True, stop=True)
                        # w = exp(prelu(al_s[src] + al_d[dst]))
                        ew = wpool.tile([128, Q * hh], dt.float32, tag="ew")
                        nc.vector.tensor_tensor(
                            out=ew[:].rearrange("p (q h) -> p q h", h=hh),
                            in0=als_view(g), in1=ps_e[:],
                            op=mybir.AluOpType.add)
                        nc.scalar.activation(ew[:], ew[:],
                                             mybir.ActivationFunctionType.Prelu,
                                             alpha=NEG)
                        nc.scalar.activation(
                            gw[:, :, hcols:mcols],
                            ew[:].rearrange("p (q h) -> p q h", h=hh),
                            mybir.ActivationFunctionType.Exp)
                        # messages: w*h (broadcast over C), head-split DVE/Pool
                        cw = hcols // hh
                        hv = h_view(g)
                        g4o = gw[:, :, 0:hcols].rearrange("p q (h c) -> p q h c", c=cw)
                        wb = gw[:, :, hcols:mcols]
                        kd = DVE_HEADS if hh > 1 else 1
                        nc.vector.tensor_tensor(
                            out=g4o[:, :, 0:kd, :], in0=hv[:, :, 0:kd, :],
                            in1=wb[:, :, 0:kd, None].to_broadcast([128, Q, kd, cw]),
                            op=mybir.AluOpType.mult)
                        if hh > kd:
                            nc.gpsimd.tensor_tensor(
                                out=g4o[:, :, kd:hh, :], in0=hv[:, :, kd:hh, :],
                                in1=wb[:, :, kd:hh, None]
                                .to_broadcast([128, Q, hh - kd, cw]),
                                op=mybir.AluOpType.mult)
                        # aggregation: one-hot matmuls with PSUM accumulation
                        ps = pp.tile([128, mcols], dt.float32, tag="agg")
                        for s in range(Q):
                            nc.tensor.matmul(
                                ps[:], sblk[:, (b * Q + s) * 128:(b * Q + s + 1) * 128],
                                gw[:, s, :], start=(s == 0), stop=(s == Q - 1))
                        # normalize + bias (+relu)
                        den = wpool.tile([128, hh], dt.float32, tag="den")
                        nc.vector.tensor_scalar_add(den[:], ps[:, hcols:mcols], 1e-16)
                        nc.vector.reciprocal(den[:], den[:])
                        if xt_out is None:
                            # final layer: normalized rows + bias -> output
                            xf = wpool.tile([128, hcols], dt.float32, tag="xf")
                            nc.vector.tensor_tensor(
                                out=xf[:], in0=ps[:, 0:hcols],
                                in1=den[:].to_broadcast([128, hcols]),
                                op=mybir.AluOpType.mult)
                            nc.vector.tensor_tensor(out=xf[:], in0=xf[:],
                                                    in1=brep[:],
                                                    op=mybir.AluOpType.add)
                            nc.sync.dma_start(
                                out=out_d[b * 128:(b + 1) * 128, :], in_=xf[:])
                            post_block(b, pp)
                            continue
                        x2 = wpool.tile([128, hcols], dt.bfloat16, tag="x2")
                        nc.vector.tensor_tensor(
                            out=x2[:].rearrange("p (h c) -> p h c", c=cw),
                            in0=ps[:, 0:hcols].rearrange("p (h c) -> p h c", c=cw),
                            in1=den[:][:, :, None].to_broadcast([128, hh, cw]),
                            op=mybir.AluOpType.mult)
                        nc.vector.tensor_tensor(out=x2[:], in0=x2[:], in1=brep[:],
                                                op=mybir.AluOpType.add)
                        if do_relu:
                            nc.vector.tensor_scalar_max(x2[:], x2[:], 0.0)
                        for hf in range(hcols // 128):
                            tp = pp.tile([128, 128], dt.bfloat16, tag="tp")
                            nc.tensor.transpose(
                                tp[:], x2[:, hf * 128:(hf + 1) * 128], ident[:])
                            nc.scalar.activation(
                                xt_out[:, hf, b * 128:(b + 1) * 128], tp[:],
                                mybir.ActivationFunctionType.Copy)
                        post_block(b, pp)

            def phase_a2(b, pp):
                """Interleaved layer-2 feature matmul + compact AG + restride."""
                ps2 = pp.tile([128, 264], dt.float32, tag="ps2")
                for k in range(2):
                    nc.tensor.matmul(ps2[:], xt2[:, k, b * 128:(b + 1) * 128],
                                     w2e[:, k, :], start=(k == 0), stop=(k == 1))
                hrow = wpool.tile([128, 256], dt.float8e4, tag="hrow2")
                nc.scalar.activation(hrow[:], ps2[:, 0:256],
                                     mybir.ActivationFunctionType.Copy)
                alrow = wpool.tile([128, 8], dt.bfloat16, tag="alrow2")
                nc.vector.tensor_copy(alrow[:], ps2[:, 256:264])
                nc.scalar.activation(alD2[:, b, :], ps2[:, 260:264],
                                     mybir.ActivationFunctionType.Copy)
                nc.sync.dma_start(out=tin2[b * 128:(b + 1) * 128, 0:256],
                                  in_=hrow[:])
                nc.sync.dma_start(out=tin2[b * 128:(b + 1) * 128, 256:272],
                                  in_=alrow[:].bitcast(dt.float8e4))
                for lo, hi, base in _regions():
                    if hi == (b + 1) * 128:
                        gn = NCORE * (hi - lo)
                        nc.gpsimd.collective_compute(
                            "AllGather", mybir.AluOpType.bypass,
                            replica_groups=[list(range(NCORE))],
                            ins=[tin2[lo:hi, :]],
                            outs=[tabc2[base:base + gn, :]])
                        nc.sync.dma_start(out=tabg2[base:base + gn, 0:CB12],
                                          in_=tabc2[base:base + gn, :])

            def phase_a3(b, pp):
                """Interleaved layer-3 feature matmul (classifier folded in:
                z = h3 @ wc rides in cols 0:4) + AG."""
                ps3 = pp.tile([128, 6], dt.float32, tag="ps2")
                for k in range(2):
                    nc.tensor.matmul(ps3[:], xt2b[:, k, b * 128:(b + 1) * 128],
                                     w3e[:, k, :], start=(k == 0), stop=(k == 1))
                hrow = wpool.tile([128, 6], dt.bfloat16, tag="hrow2")
                nc.scalar.activation(hrow[:], ps3[:],
                                     mybir.ActivationFunctionType.Copy)
                nc.scalar.activation(alD3[:, b, :], ps3[:, 5:6],
                                     mybir.ActivationFunctionType.Copy)
                nc.sync.dma_start(out=tin3[b * 128:(b + 1) * 128, :],
                                  in_=hrow[:])
                for lo, hi, base in _regions():
                    if hi == (b + 1) * 128:
                        gn = NCORE * (hi - lo)
                        nc.gpsimd.collective_compute(
                            "AllGather", mybir.AluOpType.bypass,
                            replica_groups=[list(range(NCORE))],
                            ins=[tin3[lo:hi, :]],
                            outs=[tabc3[base:base + gn, :]])
                        nc.sync.dma_start(out=tabg3[base:base + gn, 0:6],
                                          in_=tabc3[base:base + gn, :])

            # ===== edge phases
            edge_phase(
                tab1, 384, dt.bfloat16, 256, H,
                h_view=lambda g: g[:, :, 0:256].rearrange("p q (h c) -> p q h c", c=C),
                als_view=lambda g: g[:, :, 256:260],
                ald_ap=lambda b: alD1[:, b, 4:8],
                brep=b1r, do_relu=True, xt_out=xt2, post_block=phase_a2)
            edge_phase(
                tabg2, RB12, dt.float8e4, 256, H,
                h_view=lambda g: g[:, :, 0:256].rearrange("p q (h c) -> p q h c", c=C),
                als_view=lambda g: g[:, :, 256:264].bitcast(dt.bfloat16)[:, :, 0:4],
                ald_ap=lambda b: alD2[:, b, :],
                brep=b2r, do_relu=True, xt_out=xt2b, post_block=phase_a3)
            edge_phase(
                tabg3, 128, dt.bfloat16, 4, 1,
                h_view=lambda g: g[:, :, 0:4].rearrange("p q (h c) -> p q h c", c=4),
                als_view=lambda g: g[:, :, 4:5],
                ald_ap=lambda b: alD3[:, b, :],
                brep=bcr, do_relu=False, xt_out=None,
                post_block=lambda b, pp: None)

    nc.compile()
    return nc


def kernel(x, edge_index, w1, as1, ad1, b1, w2, as2, ad2, b2,
           w3, as3, ad3, b3, wc, bc):
    x = np.asarray(x)
    ei = np.asarray(edge_index).astype(np.int64)
    loop = np.arange(N, dtype=np.int64)
    src = np.concatenate([ei[0], loop])
    dst = np.concatenate([ei[1], loop])

    Q, edge_data = _prep_edges(src, dst)

    w1e = _fold_w(np.asarray(w1), np.asarray(as1), np.asarray(ad1)).astype(BF16)
    w2e = _fold_w(np.asarray(w2), np.asarray(as2), np.asarray(ad2)).astype(BF16)
    # layer 3 with classifier folded: cols = [W3@wc (4) | W3@a_s | W3@a_d]
    W364 = np.asarray(w3).astype(np.float64)
    wc64 = np.asarray(wc).astype(np.float64)
    As3 = np.asarray(as3).astype(np.float64)[0]
    Ad3 = np.asarray(ad3).astype(np.float64)[0]
    w3e = np.concatenate([W364 @ wc64, (W364 @ As3)[:, None],
                          (W364 @ Ad3)[:, None]], axis=1).astype(BF16)
    b1r = np.tile(np.asarray(b1).astype(BF16)[None, :], (128, 1))
    b2r = np.tile(np.asarray(b2).astype(BF16)[None, :], (128, 1))
    bc_f = (np.asarray(b3).astype(np.float64) @ wc64
            + np.asarray(bc).astype(np.float64)).astype(np.float32)
    bcr = np.tile(bc_f[None, :], (128, 1))

    # replicated fp8 x^T in region row order, pre-chunked
    xg = np.zeros((FIN, NG), BF16)
    xT = x.T.astype(BF16)
    for c in range(NCORE):
        xg[:, c * NPAD:c * NPAD + NLOC] = xT[:, c * NLOC:(c + 1) * NLOC]
    cc, ll = np.meshgrid(np.arange(NCORE), np.arange(NPAD), indexing="ij")
    inv = np.empty(NG, np.int64)
    inv[_rowmap(cc.ravel(), ll.ravel())] = (cc * NPAD + ll).ravel()
    xtg = np.ascontiguousarray(
        xg[:, inv].reshape(8, 128, NG // 128, 128).transpose(1, 2, 0, 3)
    ).reshape(128, -1)

    key = ("k5", Q)
    if key not in _cache:
        _cache[key] = _build(Q)
    nc = _cache[key]

    in_maps = []
    for c in range(NCORE):
        idx_s, dst_rel = edge_data[c]
        sblk, sblkT = _onehots(dst_rel, Q)
        ixd_own = _rowmap(c, np.arange(NPAD))
        in_maps.append({
            "xtg": xtg, "w1e": w1e, "w2e": w2e, "w3e": w3e,
            "b1r": b1r, "b2r": b2r, "bcr": bcr,
            "ixs": _wrap_idx(idx_s.reshape(-1)), "ixd": _wrap_idx(ixd_own),
            "sblk": sblk, "sblkT": sblkT,
        })
    res = run_bass_kernel_spmd(nc, in_maps, core_ids=list(range(NCORE)),
                               tmpdir=os.environ.get("BASS_TMPDIR") or None)
    global LAST_RESULTS
    LAST_RESULTS = res
    out = np.concatenate([res.results[c]["out"][:NLOC] for c in range(NCORE)],
                         axis=0)
    return out.astype(np.float32)


# revision 59
# speedup vs baseline: 1.0449x; 1.0449x over previous
"""3-layer GAT on 8 TRN2 NeuronCores (v3).

Node-sharded 8 ways (2500/core padded to 2560). Per layer, a global node
table holds [h | al_s | al_d] rows; edges (sorted by dst block, padded to
Q*128 per block) gather src rows and aggregate via one-hot matmuls.

- Layer-1 table: replicated compute (no collective) — every core runs the
  feature matmul for all 160 global chunks from the replicated fp8 x input.
- Tables are fp8 rows (h fp8, al bf16 packed in trailing bytes): 512B rows
  for layers 1/2 (256 fp8 h + 16B al), 256B rows for layer 3 (64 bf16 h +
  4B al). Gathers fetch one row per edge.
- al_d per edge: Q mini-matmuls per block against host-precomputed
  transposed one-hot chunks (fp8, SBUF-cached; stationary loads are free).
- Aggregation: per 128-edge chunk, matmul(one-hot fp8, [w*h | w] bf16) with
  PSUM accumulation; segment softmax denominator rides in the last cols.
- Layers 2/3 tables: next-layer feature matmul interleaved per-block into
  the edge phase; rows AllGathered compactly in 3 regions (region-major
  global layout keeps outputs contiguous), then re-strided locally to
  gather-legal 256B-multiple row pitch. Collectives overlap edge compute.
"""
import os
import sys
sys.path.insert(0, "/opt/trn_rl_repo")
import numpy as np
import ml_dtypes

import concourse.bass as bass
import concourse.tile as tile
from concourse import bacc, mybir
from concourse.bass_utils import run_bass_kernel_spmd
from concourse.masks import make_identity

BF16 = ml_dtypes.bfloat16
FP8 = ml_dtypes.float8_e4m3fn
N = 20000
E = 320000
FIN = 1024
H = 4
C = 64
NEG = 0.2
NCORE = 8
NLOC = 2500
NPAD = 2560           # per-core node rows, padded to x128
NBLK = NPAD // 128    # dst blocks per core
NG = NCORE * NPAD     # global table rows
GBLK = NG // 128      # global 128-chunks
RB12 = 512            # L1/L2 row bytes: h fp8 0:256, al bf16 256:272
CB12 = 272            # compact row bytes (AllGather payload)
RB3 = 256             # L3 row bytes: h bf16 0:128, al bf16 128:132
CB3 = 132
DVE_HEADS = 4         # heads 0:k multiplied on DVE, rest on GPSIMD
AG_SPLITS = [640, 1280, 1920]  # chunked-AllGather region bounds (local rows)

_cache = {}


def _regions():
    bounds = [0] + AG_SPLITS + [NPAD]
    regs = []
    base = 0
    for lo, hi in zip(bounds[:-1], bounds[1:]):
        regs.append((lo, hi, base))
        base += NCORE * (hi - lo)
    return regs


def _rowmap(core, loc):
    """Global table row id for node (core, loc) under the region layout
    [region, core, rows] — keeps chunked AllGather outputs contiguous."""
    core, loc = np.broadcast_arrays(np.asarray(core, np.int64),
                                    np.asarray(loc, np.int64))
    out = np.empty_like(loc)
    for lo, hi, base in _regions():
        m = (loc >= lo) & (loc < hi)
        out[m] = base + core[m] * (hi - lo) + (loc[m] - lo)
    return out


def _prep_edges(src, dst):
    """Partition edges by dst core, sort by (dst block, src), pad per block.

    Returns (Q, per_core) with per_core[c] = (idx_s [NBLK, Q*128] global row
    ids, dst_rel [NBLK, Q*128] int32, -1 for pad)."""
    core = dst // NLOC
    dloc = dst - core * NLOC
    blk = dloc // 128
    per_core_lists = []
    maxq = 0
    for c in range(NCORE):
        m = core == c
        s_c, dl_c, b_c = src[m], dloc[m], blk[m]
        order = np.lexsort((s_c, b_c))
        s_c, dl_c, b_c = s_c[order], dl_c[order], b_c[order]
        counts = np.bincount(b_c, minlength=NBLK)
        maxq = max(maxq, int(np.ceil(counts.max() / 128)))
        per_core_lists.append((s_c, dl_c, counts))
    Q = maxq
    EPB = Q * 128
    out = []
    for c in range(NCORE):
        s_c, dl_c, counts = per_core_lists[c]
        idx_s = np.zeros((NBLK, EPB), np.int64)
        dst_rel = np.full((NBLK, EPB), -1, np.int32)
        pos = 0
        for b in range(NBLK):
            n = counts[b]
            sb = s_c[pos:pos + n]
            db = dl_c[pos:pos + n]
            pos += n
            idx_s[b, :n] = _rowmap(sb // NLOC, sb % NLOC)
            dst_rel[b, :n] = db % 128
        out.append((idx_s, dst_rel))
    return Q, out


def _wrap_idx(idx_flat):
    """flat [n] -> dma_gather idx layout [128, n//16] int16."""
    n = idx_flat.shape[0]
    w = idx_flat.reshape(n // 16, 16).T.astype(np.int16)
    return np.tile(w, (8, 1))


def _onehots(dst_rel, Q):
    """dst_rel [NBLK, Q*128] -> (sblk, sblkT) [128, NBLK*Q*128] fp8."""
    r = dst_rel.reshape(NBLK, Q, 128)              # [b, s, p]
    eye = np.eye(128, dtype=FP8)
    lut = np.concatenate([eye, np.zeros((1, 128), FP8)], axis=0)
    oh = lut[np.where(r < 0, 128, r)]              # [b, s, p, d]
    sblk = np.ascontiguousarray(oh.transpose(2, 0, 1, 3)).reshape(128, -1)
    sblkT = np.ascontiguousarray(oh.transpose(3, 0, 1, 2)).reshape(128, -1)
    return sblk, sblkT


def _fold_w(W, a_s, a_d):
    """[F, H*C] weights + per-head a vectors -> [F, H*C + 2H] f32."""
    F = W.shape[0]
    Hh, Cc = a_s.shape
    As = np.zeros((Hh * Cc, Hh), np.float64)
    Ad = np.zeros((Hh * Cc, Hh), np.float64)
    for h in range(Hh):
        As[h * Cc:(h + 1) * Cc, h] = a_s[h]
        Ad[h * Cc:(h + 1) * Cc, h] = a_d[h]
    W64 = W.astype(np.float64)
    return np.concatenate([W64, W64 @ As, W64 @ Ad], axis=1).astype(np.float32)


def _build(Q):
    dt = mybir.dt
    nc = bacc.Bacc("TRN2", num_devices=NCORE, debug=False, num_swdge_queues=4)
    NBQ = NBLK * Q

    # x^T replicated, fp8, pre-chunked: [p, (g k n)] = xT[k*128+p, col(g*128+n)]
    xtg_in = nc.dram_tensor("xtg", [128, GBLK * 8 * 128], dt.bfloat16,
                            kind="ExternalInput")
    w1e_in = nc.dram_tensor("w1e", [FIN, 264], dt.bfloat16, kind="ExternalInput")
    w2e_in = nc.dram_tensor("w2e", [256, 264], dt.bfloat16, kind="ExternalInput")
    w3e_in = nc.dram_tensor("w3e", [256, 6], dt.bfloat16, kind="ExternalInput")

    b1_in = nc.dram_tensor("b1t", [128, 2], dt.bfloat16, kind="ExternalInput")
    b2_in = nc.dram_tensor("b2t", [128, 2], dt.bfloat16, kind="ExternalInput")

    bc_in = nc.dram_tensor("bcr", [128, 4], dt.float32, kind="ExternalInput")
    ixs_in = nc.dram_tensor("ixs", [128, NBQ * 8], dt.int16, kind="ExternalInput")
    ixd_in = nc.dram_tensor("ixd", [128, NPAD // 16], dt.int16, kind="ExternalInput")
    sblk_in = nc.dram_tensor("sblk", [128, NBQ * 128], dt.float8e4,
                             kind="ExternalInput")
    sblkT_in = nc.dram_tensor("sblkT", [128, NBQ * 128], dt.float8e4,
                              kind="ExternalInput")
    out_d = nc.dram_tensor("out", [NPAD, 4], dt.float32, kind="ExternalOutput")

    tab1 = nc.dram_tensor("tab1", [NG, 384], dt.bfloat16, kind="Internal")
    tin2 = nc.dram_tensor("tin2", [NPAD, CB12], dt.float8e4, kind="Internal")
    tabc2 = nc.dram_tensor("tabc2", [NG, CB12], dt.float8e4, kind="Internal",
                           addr_space="Shared")
    tabg2 = nc.dram_tensor("tabg2", [NG, RB12], dt.float8e4, kind="Internal")
    tin3 = nc.dram_tensor("tin3", [NPAD, 6], dt.bfloat16, kind="Internal")
    tabc3 = nc.dram_tensor("tabc3", [NG, 6], dt.bfloat16, kind="Internal",
                           addr_space="Shared")
    tabg3 = nc.dram_tensor("tabg3", [NG, 128], dt.bfloat16, kind="Internal")

    with tile.TileContext(nc) as tc:
        with (
            tc.tile_pool(name="const", bufs=1) as cpool,
            tc.tile_pool(name="work", bufs=2) as wpool,
        ):
            # ---- constants to SBUF
            ident = cpool.tile([128, 128], dt.bfloat16)
            make_identity(nc, ident[:])
            ixs = cpool.tile([128, NBQ * 8], dt.int16)
            ixd = cpool.tile([128, NPAD // 16], dt.int16)
            nc.sync.dma_start(out=ixd[:], in_=ixd_in[:])
            sblk = cpool.tile([128, NBQ * 128], dt.float8e4)
            sblkT8 = cpool.tile([128, NBQ * 128], dt.float8e4)
            w1e = cpool.tile([128, 8, 264], dt.bfloat16)
            nc.sync.dma_start(out=w1e[:], in_=w1e_in[:].rearrange("(k p) c -> p k c", p=128))
            w2e = cpool.tile([128, 2, 264], dt.bfloat16)
            nc.sync.dma_start(out=w2e[:], in_=w2e_in[:].rearrange("(k p) c -> p k c", p=128))
            w3e = cpool.tile([128, 2, 6], dt.bfloat16)
            nc.sync.dma_start(out=w3e[:], in_=w3e_in[:].rearrange("(k p) c -> p k c", p=128))

            b1t = cpool.tile([128, 2], dt.bfloat16)
            nc.sync.dma_start(out=b1t[:], in_=b1_in[:])
            b2t = cpool.tile([128, 2], dt.bfloat16)
            nc.sync.dma_start(out=b2t[:], in_=b2_in[:])

            bcr = cpool.tile([128, 4], dt.float32)
            nc.sync.dma_start(out=bcr[:], in_=bc_in[:])

            alD1 = cpool.tile([128, NBLK, 128], dt.bfloat16)
            alD2 = cpool.tile([128, NBLK, H], dt.bfloat16)
            alD3 = cpool.tile([128, NBLK, 1], dt.bfloat16)
            xt2 = cpool.tile([128, 2, NPAD], dt.bfloat16)
            xt2b = xt2  # lifetimes don't overlap; tile deps order the reuse


            # ===== layer 1: replicated phase_a, batched x4
            GB = 4
            with tc.tile_pool(name="psA", bufs=2, space="PSUM") as ppA:
                for gg in range(0, GBLK, GB):
                    xc = wpool.tile([128, GB, 8, 128], dt.bfloat16, tag="xc", bufs=3)
                    nc.sync.dma_start(
                        out=xc[:],
                        in_=xtg_in[:, gg * 1024:(gg + GB) * 1024]
                        .rearrange("p (j k n) -> p j k n", k=8, n=128))
                    hrow = wpool.tile([128, GB, 264], dt.bfloat16, tag="hrow")
                    for j in range(GB):
                        ps = ppA.tile([128, 264], dt.float32, tag=f"psA{j % 2}")
                        for k in range(8):
                            nc.tensor.matmul(ps[:], xc[:, j, k, :], w1e[:, k, :],
                                             start=(k == 0), stop=(k == 7))
                        if j % 2 == 0:
                            nc.scalar.activation(hrow[:, j, :], ps[:],
                                                 mybir.ActivationFunctionType.Copy)
                        else:
                            nc.vector.tensor_copy(hrow[:, j, :], ps[:])
                    nc.sync.dma_start(
                        out=tab1[gg * 128:(gg + GB) * 128, 0:264]
                        .rearrange("(j p) c -> p j c", p=128),
                        in_=hrow[:])
            # one-hot caches + edge indices load under the L1 compute
            nc.sync.dma_start(out=ixs[:], in_=ixs_in[:])
            nc.sync.dma_start(out=sblk[:], in_=sblk_in[:])
            nc.sync.dma_start(out=sblkT8[:], in_=sblkT_in[:])
            # own nodes' rows for al_d: one gather
            nc.gpsimd.dma_gather(
                out_ap=alD1[:], in_ap=tab1[:, 256:384], idxs_ap=ixd[:],
                num_idxs=NPAD, num_idxs_reg=NPAD, elem_size=128,
                elem_step=384, single_packet=False, queue_num=0)

            def edge_phase(tab, rb, tdt, hcols, hh, h_view, als_view, ald_ap,
                           brep, do_relu, xt_out, post_block):
                """Per-dst-block edge processing. h_view/als_view map the raw
                gathered fp8-byte tile to h and al_s APs; ald_ap(b) gives the
                block's own al_d [128, hh] bf16."""
                mcols = hcols + hh
                with tc.tile_pool(name="psE", bufs=2, space="PSUM") as pp:
                    for b in range(NBLK):
                        g = wpool.tile([128, Q, rb], tdt, tag="g", bufs=3)
                        nc.gpsimd.dma_gather(
                            out_ap=g[:], in_ap=tab[:],
                            idxs_ap=ixs[:, b * Q * 8:(b + 1) * Q * 8],
                            num_idxs=Q * 128, num_idxs_reg=Q * 128,
                            elem_size=rb, single_packet=False,
                            queue_num=b % 4)
                        gw = wpool.tile([128, Q, mcols], dt.bfloat16, tag="gw")
                        # al_d per edge via mini-matmuls vs transposed one-hots
                        ps_e = pp.tile([128, Q, hh], dt.float32, tag="ps_e")
                        for s in range(Q):
                            nc.tensor.matmul(
                                ps_e[:, s, :],
                                sblkT8[:, (b * Q + s) * 128:(b * Q + s + 1) * 128],
                                ald_ap(b), start=True, stop=True)
                        # w = exp(prelu(al_s[src] + al_d[dst]))
                        ew = wpool.tile([128, Q * hh], dt.float32, tag="ew", bufs=3)
                        nc.vector.tensor_tensor(
                            out=ew[:].rearrange("p (q h) -> p q h", h=hh),
                            in0=als_view(g), in1=ps_e[:],
                            op=mybir.AluOpType.add)
                        nc.scalar.activation(ew[:], ew[:],
                                             mybir.ActivationFunctionType.Prelu,
                                             alpha=NEG)
                        nc.scalar.activation(
                            gw[:, :, hcols:mcols],
                            ew[:].rearrange("p (q h) -> p q h", h=hh),
                            mybir.ActivationFunctionType.Exp)
                        # messages: w*h (broadcast over C), head-split DVE/Pool
                        cw = hcols // hh
                        hv = h_view(g)
                        g4o = gw[:, :, 0:hcols].rearrange("p q (h c) -> p q h c", c=cw)
                        wb = gw[:, :, hcols:mcols]
                        kd = DVE_HEADS if hh > 1 else 1
                        nc.vector.tensor_tensor(
                            out=g4o[:, :, 0:kd, :], in0=hv[:, :, 0:kd, :],
                            in1=wb[:, :, 0:kd, None].to_broadcast([128, Q, kd, cw]),
                            op=mybir.AluOpType.mult)
                        if hh > kd:
                            nc.gpsimd.tensor_tensor(
                                out=g4o[:, :, kd:hh, :], in0=hv[:, :, kd:hh, :],
                                in1=wb[:, :, kd:hh, None]
                                .to_broadcast([128, Q, hh - kd, cw]),
                                op=mybir.AluOpType.mult)
                        # aggregation: one-hot matmuls with PSUM accumulation
                        ps = pp.tile([128, mcols], dt.float32, tag="agg")
                        for s in range(Q):
                            nc.tensor.matmul(
                                ps[:], sblk[:, (b * Q + s) * 128:(b * Q + s + 1) * 128],
                                gw[:, s, :], start=(s == 0), stop=(s == Q - 1))
                        # normalize + bias (+relu)
                        den = wpool.tile([128, hh], dt.float32, tag="den", bufs=3)
                        nc.vector.tensor_scalar_add(den[:], ps[:, hcols:mcols], 1e-16)
                        nc.vector.reciprocal(den[:], den[:])
                        if xt_out is None:
                            # final layer: normalized rows + bias -> output
                            xf = wpool.tile([128, hcols], dt.float32, tag="xf")
                            nc.vector.tensor_tensor(
                                out=xf[:], in0=ps[:, 0:hcols],
                                in1=den[:].to_broadcast([128, hcols]),
                                op=mybir.AluOpType.mult)
                            nc.vector.tensor_tensor(out=xf[:], in0=xf[:],
                                                    in1=brep[:],
                                                    op=mybir.AluOpType.add)
                            nc.sync.dma_start(
                                out=out_d[b * 128:(b + 1) * 128, :], in_=xf[:])
                            post_block(b, pp)
                            continue
                        x2 = wpool.tile([128, hcols], dt.bfloat16, tag="x2", bufs=3)
                        nc.vector.tensor_tensor(
                            out=x2[:].rearrange("p (h c) -> p h c", c=cw),
                            in0=ps[:, 0:hcols].rearrange("p (h c) -> p h c", c=cw),
                            in1=den[:][:, :, None].to_broadcast([128, hh, cw]),
                            op=mybir.AluOpType.mult)
                        # bias + relu ride the post-transpose copy: features
                        # sit on partitions there, so the per-feature bias is
                        # a per-partition Act bias; relu commutes w/ transpose
                        for hf in range(hcols // 128):
                            tp = pp.tile([128, 128], dt.bfloat16, tag="tp")
                            nc.tensor.transpose(
                                tp[:], x2[:, hf * 128:(hf + 1) * 128], ident[:])
                            nc.scalar.activation(
                                xt_out[:, hf, b * 128:(b + 1) * 128], tp[:],
                                mybir.ActivationFunctionType.Relu,
                                bias=brep[:, hf:hf + 1])
                        post_block(b, pp)

            def phase_a2(b, pp):
                """Interleaved layer-2 feature matmul + compact AG + restride."""
                ps2 = pp.tile([128, 264], dt.float32, tag="ps2")
                for k in range(2):
                    nc.tensor.matmul(ps2[:], xt2[:, k, b * 128:(b + 1) * 128],
                                     w2e[:, k, :], start=(k == 0), stop=(k == 1))
                hrow = wpool.tile([128, 256], dt.float8e4, tag="hrow2")
                nc.scalar.activation(hrow[:], ps2[:, 0:256],
                                     mybir.ActivationFunctionType.Copy)
                alrow = wpool.tile([128, 8], dt.bfloat16, tag="alrow2")
                nc.vector.tensor_copy(alrow[:], ps2[:, 256:264])
                nc.scalar.activation(alD2[:, b, :], ps2[:, 260:264],
                                     mybir.ActivationFunctionType.Copy)
                nc.sync.dma_start(out=tin2[b * 128:(b + 1) * 128, 0:256],
                                  in_=hrow[:])
                nc.sync.dma_start(out=tin2[b * 128:(b + 1) * 128, 256:272],
                                  in_=alrow[:].bitcast(dt.float8e4))
                for lo, hi, base in _regions():
                    if hi == (b + 1) * 128:
                        gn = NCORE * (hi - lo)
                        nc.gpsimd.collective_compute(
                            "AllGather", mybir.AluOpType.bypass,
                            replica_groups=[list(range(NCORE))],
                            ins=[tin2[lo:hi, :]],
                            outs=[tabc2[base:base + gn, :]])
                        nc.sync.dma_start(out=tabg2[base:base + gn, 0:CB12],
                                          in_=tabc2[base:base + gn, :])

            def phase_a3(b, pp):
                """Interleaved layer-3 feature matmul (classifier folded in:
                z = h3 @ wc rides in cols 0:4) + AG."""
                ps3 = pp.tile([128, 6], dt.float32, tag="ps2")
                for k in range(2):
                    nc.tensor.matmul(ps3[:], xt2b[:, k, b * 128:(b + 1) * 128],
                                     w3e[:, k, :], start=(k == 0), stop=(k == 1))
                hrow = wpool.tile([128, 6], dt.bfloat16, tag="hrow2")
                nc.scalar.activation(hrow[:], ps3[:],
                                     mybir.ActivationFunctionType.Copy)
                nc.scalar.activation(alD3[:, b, :], ps3[:, 5:6],
                                     mybir.ActivationFunctionType.Copy)
                nc.sync.dma_start(out=tin3[b * 128:(b + 1) * 128, :],
                                  in_=hrow[:])
                for lo, hi, base in _regions():
                    if hi == (b + 1) * 128:
                        gn = NCORE * (hi - lo)
                        nc.gpsimd.collective_compute(
                            "AllGather", mybir.AluOpType.bypass,
                            replica_groups=[list(range(NCORE))],
                            ins=[tin3[lo:hi, :]],
                            outs=[tabc3[base:base + gn, :]])
                        nc.sync.dma_start(out=tabg3[base:base + gn, 0:6],
                                          in_=tabc3[base:base + gn, :])

            # ===== edge phases
            edge_phase(
                tab1, 384, dt.bfloat16, 256, H,
                h_view=lambda g: g[:, :, 0:256].rearrange("p q (h c) -> p q h c", c=C),
                als_view=lambda g: g[:, :, 256:260],
                ald_ap=lambda b: alD1[:, b, 4:8],
                brep=b1t, do_relu=True, xt_out=xt2, post_block=phase_a2)
            edge_phase(
                tabg2, RB12, dt.float8e4, 256, H,
                h_view=lambda g: g[:, :, 0:256].rearrange("p q (h c) -> p q h c", c=C),
                als_view=lambda g: g[:, :, 256:264].bitcast(dt.bfloat16)[:, :, 0:4],
                ald_ap=lambda b: alD2[:, b, :],
                brep=b2t, do_relu=True, xt_out=xt2b, post_block=phase_a3)
            edge_phase(
                tabg3, 128, dt.bfloat16, 4, 1,
                h_view=lambda g: g[:, :, 0:4].rearrange("p q (h c) -> p q h c", c=4),
                als_view=lambda g: g[:, :, 4:5],
                ald_ap=lambda b: alD3[:, b, :],
                brep=bcr, do_relu=False, xt_out=None,
                post_block=lambda b, pp: None)

    nc.compile()
    return nc


def kernel(x, edge_index, w1, as1, ad1, b1, w2, as2, ad2, b2,
           w3, as3, ad3, b3, wc, bc):
    x = np.asarray(x)
    ei = np.asarray(edge_index).astype(np.int64)
    loop = np.arange(N, dtype=np.int64)
    src = np.concatenate([ei[0], loop])
    dst = np.concatenate([ei[1], loop])

    Q, edge_data = _prep_edges(src, dst)

    w1e = _fold_w(np.asarray(w1), np.asarray(as1), np.asarray(ad1)).astype(BF16)
    w2e = _fold_w(np.asarray(w2), np.asarray(as2), np.asarray(ad2)).astype(BF16)
    # layer 3 with classifier folded: cols = [W3@wc (4) | W3@a_s | W3@a_d]
    W364 = np.asarray(w3).astype(np.float64)
    wc64 = np.asarray(wc).astype(np.float64)
    As3 = np.asarray(as3).astype(np.float64)[0]
    Ad3 = np.asarray(ad3).astype(np.float64)[0]
    w3e = np.concatenate([W364 @ wc64, (W364 @ As3)[:, None],
                          (W364 @ Ad3)[:, None]], axis=1).astype(BF16)
    b1t = np.asarray(b1).astype(BF16).reshape(2, 128).T.copy()
    b2t = np.asarray(b2).astype(BF16).reshape(2, 128).T.copy()
    bc_f = (np.asarray(b3).astype(np.float64) @ wc64
            + np.asarray(bc).astype(np.float64)).astype(np.float32)
    bcr = np.tile(bc_f[None, :], (128, 1))

    # replicated fp8 x^T in region row order, pre-chunked
    xg = np.zeros((FIN, NG), BF16)
    xT = x.T.astype(BF16)
    for c in range(NCORE):
        xg[:, c * NPAD:c * NPAD + NLOC] = xT[:, c * NLOC:(c + 1) * NLOC]
    cc, ll = np.meshgrid(np.arange(NCORE), np.arange(NPAD), indexing="ij")
    inv = np.empty(NG, np.int64)
    inv[_rowmap(cc.ravel(), ll.ravel())] = (cc * NPAD + ll).ravel()
    xtg = np.ascontiguousarray(
        xg[:, inv].reshape(8, 128, NG // 128, 128).transpose(1, 2, 0, 3)
    ).reshape(128, -1)

    key = ("k7", Q)
    if key not in _cache:
        _cache[key] = _build(Q)
    nc = _cache[key]

    in_maps = []
    for c in range(NCORE):
        idx_s, dst_rel = edge_data[c]
        sblk, sblkT = _onehots(dst_rel, Q)
        ixd_own = _rowmap(c, np.arange(NPAD))
        in_maps.append({
            "xtg": xtg, "w1e": w1e, "w2e": w2e, "w3e": w3e,
            "b1t": b1t, "b2t": b2t, "bcr": bcr,
            "ixs": _wrap_idx(idx_s.reshape(-1)), "ixd": _wrap_idx(ixd_own),
            "sblk": sblk, "sblkT": sblkT,
        })
    res = run_bass_kernel_spmd(nc, in_maps, core_ids=list(range(NCORE)),
                               tmpdir=os.environ.get("BASS_TMPDIR") or None)
    global LAST_RESULTS
    LAST_RESULTS = res
    out = np.concatenate([res.results[c]["out"][:NLOC] for c in range(NCORE)],
                         axis=0)
    return out.astype(np.float32)
